# revision 76
# baseline (speedup 1.0000x reference)
"""GNN message-passing (NBFNet-style) Trainium2 kernel: host prep + Bass/Tile builder.

Layout strategy (per core, 2 batches):
  - partition dim = 128 = (batch_local in {0,1}) x (feature d in 0..63)
  - node-state tensors [128, NPAD] in "rank space" (nodes sorted by in-degree desc, per snapshot)
  - edge messages gathered via GPSIMD dma_gather (transpose mode) from HBM node-major
    tables [NPAD, 128] bf16; HBM row order uses a per-snap permutation R so table
    writebacks are 5120B-contiguous per partition (no <512B DMA penalty)
  - segment stats (sum/sqsum/max/min) via degree-sorted "rounds": round r updates the
    prefix of ranks with indeg >= r, as in-place DVE tensor_tensor ops (bf16, 2x mode);
    round 0 "first-touches" the accumulators from the boundary tile (no init copies)
  - snap0 layer0 shortcut: boundary is a one-hot indicator, so all but ~30 "special"
    columns reduce to relu(u0 + scl*u1 + iscl*u2) = a K=3 matmul over per-node scales;
    special columns are host-precomputed and merged via copy_predicated (per-core data)
  - snap1 initial = (indicator + out0)/2: tabB is stored pre-halved (relu scale fold),
    the indicator is scatter-added into the h rows, making tabB == init1 exactly
  - PNA combine matmul: 13 K=64 chunks packed as block-diagonal K=128 lhsT (both batches
    in one matmul), rhs = stat/product tensors, PSUM accumulation, ACT relu evacuation
"""
import os
import sys
import contextlib

import numpy as np

sys.path.insert(0, "/opt/trn_rl_repo")
import ml_dtypes  # noqa: E402
import concourse.bass as bass  # noqa: E402
import concourse.tile as tile  # noqa: E402
from concourse import bacc, mybir, bass_utils, library_config  # noqa: E402
from concourse.tile import add_dep_helper  # noqa: E402


def _dep(post, pre, reason):
    if pre is not None:
        add_dep_helper(post.ins, pre.ins, reason=reason)

BF16 = mybir.dt.bfloat16
F32 = mybir.dt.float32
I16 = mybir.dt.int16
AF = mybir.ActivationFunctionType
OP = mybir.AluOpType
nbf = ml_dtypes.bfloat16

N = 5000
NPAD = 5120
E = 30000
D = 64
B = 16
NCORES = 8
BL = B // NCORES  # 2
HIST = 2
NL = 2
NEG = 32
NREL2 = 400
EPS = 1e-6
HNP = NPAD // 2  # 2560, node-half width
CH = 3584        # edge gather chunk size (cols), multiple of 128
NTILE = 512      # matmul node tile
EPS_CLIP_SCALE = 1e-2
TPH = HNP // 128  # 20 rows per partition in permuted table order

# stat order matches reference feats: [mean, max, min, std]
STATS = ("mean", "max", "min", "std")


def _ceil(a, m):
    return (a + m - 1) // m * m


def _wrap16(idx):
    """[L] int -> [128, L/16] int16 wrapped in 16 partitions, replicated x8."""
    L = len(idx)
    assert L % 16 == 0
    w = np.asarray(idx, np.int64).reshape(L // 16, 16).T.astype(np.int16)
    return np.tile(w, (8, 1))


def _row_perm():
    """rank (within full NPAD) -> HBM table row, making transposed writebacks
    contiguous: row = h*HNP + p*TPH + t for rank = h*HNP + t*128 + p."""
    r = np.arange(NPAD)
    h = r // HNP
    c = r - h * HNP
    p, t = c % 128, c // 128
    return h * HNP + p * TPH + t


ROWP = _row_perm()  # rank -> table row


def prep_snap(src, dst, et):
    """Host index preprocessing for one snapshot."""
    src = np.asarray(src, np.int64)
    dst = np.asarray(dst, np.int64)
    et = np.asarray(et, np.int64)
    indeg = np.bincount(dst, minlength=N).astype(np.int64)
    order = np.argsort(-indeg, kind="stable")  # rank -> node
    rank_of = np.empty(N, np.int64)
    rank_of[order] = np.arange(N)
    row_of = ROWP[rank_of]  # node -> HBM table row

    er = rank_of[dst]
    eord = np.argsort(er, kind="stable")  # edges sorted by dst rank
    er_s = er[eord]
    starts = np.searchsorted(er_s, er_s, side="left")
    slot = np.arange(E) - starts  # slot within dst group (0-indexed round)
    Rmax = int(indeg.max())
    W = np.array([int(np.count_nonzero(indeg >= r)) for r in range(1, Rmax + 1)],
                 np.int64)  # W[s0] = width of round s0 (0-ind)

    # per-half round widths and offsets
    w_h = [np.clip(W - h * HNP, 0, HNP) for h in (0, 1)]
    off_h = [np.concatenate([[0], np.cumsum(w)]) for w in w_h]
    len_h = [int(off_h[0][-1]), int(off_h[1][-1])]
    len_hp = [_ceil(len_h[0], 128) if len_h[0] else 0, _ceil(len_h[1], 128) if len_h[1] else 0]
    base = [0, len_hp[0]]
    L = len_hp[0] + len_hp[1]

    half = er_s // HNP
    jloc = er_s - half * HNP
    offarr = np.stack([off_h[0][:Rmax], off_h[1][:Rmax]])  # [2, Rmax]
    basearr = np.array(base)
    pos = basearr[half] + offarr[half, slot] + jloc
    assert len(np.unique(pos)) == E
    src_rm = np.zeros(L, np.int64)
    et_rm = np.zeros(L, np.int64)
    src_rm[pos] = src[eord]
    et_rm[pos] = et[eord]

    # chunk + rounds op lists: per half, list of chunks;
    # per chunk: (c0, c1, ops) where ops = [(msg_off_in_chunk, acc_off, width, round)]
    chunks = [[], []]
    for h in (0, 1):
        c0 = base[h]
        hend = base[h] + len_hp[h]
        while c0 < hend:
            c1 = min(c0 + CH, hend)
            ops = []
            for r in range(Rmax):
                w = int(w_h[h][r])
                if w == 0:
                    continue
                g0 = base[h] + int(off_h[h][r])
                g1 = g0 + w
                a, b_ = max(g0, c0), min(g1, c1)
                if a < b_:
                    ops.append((a - c0, a - g0, b_ - a, r))
            chunks[h].append((c0, c1, ops))
            c0 = c1
    nchunks = len(chunks[0]) + len(chunks[1])

    deg = (indeg + 1).astype(np.float64)
    scl = np.log(deg)
    scl = scl / scl.mean()
    iscl = 1.0 / np.clip(scl, EPS_CLIP_SCALE, None)
    invdeg = 1.0 / deg

    def pad_rank(x, fill):
        out = np.full(NPAD, fill, np.float64)
        out[:N] = x[order]
        return out

    v = np.stack([pad_rank(invdeg, 1.0), pad_rank(scl, 1.0), pad_rank(iscl, 1.0)])
    # round-0 width per half: suffix [w0h, HNP) = indeg-0 nodes (deg==1)
    w0h = [int(w_h[0][0]) if Rmax > 0 else 0, int(w_h[1][0]) if Rmax > 0 else 0]

    return dict(
        indeg=indeg, order=order, rank_of=rank_of, row_of=row_of,
        L=L, src_rm=src_rm, et_rm=et_rm,
        chunks=chunks, v=v.astype(nbf), w0h=w0h,
        scl=scl, iscl=iscl,
    )


def _l0_special_cols(sn, h_nodes, qvecs, rel0, W0, b0, src, dst, et):
    """Exact snap0-layer0 output columns for 'special' nodes (h nodes and
    destinations of edges sourced at h). Returns (positions[rank cols], vals[128,K])."""
    indeg = sn["indeg"]
    scl, iscl = sn["scl"], sn["iscl"]
    special = set()
    sp_edges = {}  # (node, b) -> list of msg vectors
    for b in range(BL):
        hb = int(h_nodes[b])
        special.add(hb)
        mask = src == hb
        for e in np.nonzero(mask)[0]:
            n = int(dst[e])
            special.add(n)
            sp_edges.setdefault((n, b), []).append(qvecs[b] * rel0[int(et[e])])
    nodes = sorted(special)
    K = len(nodes)
    vals = np.zeros((128, K), np.float32)
    for j, n in enumerate(nodes):
        for b in range(BL):
            init_n = qvecs[b] if n == int(h_nodes[b]) else np.zeros(D, np.float32)
            sps = sp_edges.get((n, b), [])
            n_zero = indeg[n] - len(sps)
            cand = list(sps) + [init_n]
            if n_zero > 0:
                cand.append(np.zeros(D, np.float32))
            cand = np.stack(cand)
            deg = indeg[n] + 1
            s = np.sum(np.stack(sps), axis=0) if sps else np.zeros(D, np.float32)
            s = s + init_n
            mean = s / deg
            sqm = (np.sum(np.stack(sps) ** 2, axis=0) if sps else 0.0)
            sqm = (sqm + init_n ** 2) / deg
            mx = cand.max(axis=0)
            mn = cand.min(axis=0)
            std = np.sqrt(np.clip(sqm - mean ** 2, EPS, None))
            feats = np.stack([mean, mx, mn, std], axis=-1)  # [D, 4]
            scales = np.array([1.0, scl[n], iscl[n]])
            agg = (feats[:, :, None] * scales[None, None, :]).reshape(D * 12)
            upd = np.concatenate([init_n, agg]) @ W0 + b0
            vals[b * D:(b + 1) * D, j] = np.maximum(upd, 0.0)
    pos = [int(sn["rank_of"][n]) for n in nodes]
    return pos, vals


def preprocess(inputs):
    qt = np.asarray(inputs["query_triple"], np.int64)  # [B, NEG, 3]
    h_index, r_index, t_index = qt[..., 0], qt[..., 1], qt[..., 2]
    is_t_neg = np.all(h_index == h_index[:, :1], axis=-1, keepdims=True)
    h_i = np.where(is_t_neg, h_index, t_index)
    t_i = np.where(is_t_neg, t_index, h_index)
    r_i = np.where(is_t_neg, r_index, r_index + NREL2 // 2)

    ei = np.asarray(inputs["edge_index"], np.int64)
    etp = np.asarray(inputs["edge_type"], np.int64)
    snaps = [prep_snap(ei[s, 0], ei[s, 1], etp[s]) for s in range(HIST)]
    # table row spaces: tabA, tabB: row order R0; tabC, tabD: row order R1.
    r0w, r1w = snaps[0]["row_of"], snaps[1]["row_of"]
    # s0-l1 gathers tabA rows; s1-l0 gathers tabB rows; s1-l1 gathers tabC rows
    snaps[0]["xidxB_w"] = _wrap16(r0w[snaps[0]["src_rm"]])
    snaps[1]["xidxA_w"] = _wrap16(r0w[snaps[1]["src_rm"]])
    snaps[1]["xidxB_w"] = _wrap16(r1w[snaps[1]["src_rm"]])
    snaps[0]["etidx_w"] = _wrap16(snaps[0]["et_rm"])
    snaps[1]["etidx_w"] = _wrap16(snaps[1]["et_rm"])
    # snap1 initial: rank1-ordered columns gathered from tabB (row order R0)
    perm1 = np.zeros(NPAD, np.int64)
    perm1[:N] = snaps[1]["order"]
    snaps[1]["perm_w"] = _wrap16(r0w[perm1])

    qw = np.asarray(inputs["query_weight"], np.float32)      # [NREL2, D]
    rel = np.asarray(inputs["rel_embs"], np.float32)         # [NL, NREL2, D]
    lw = np.asarray(inputs["layer_Ws"], np.float32)          # [NL, 13*D, D]
    lb = np.asarray(inputs["layer_bs"], np.float32)          # [NL, D]
    w1 = np.asarray(inputs["mlp_w1"], np.float32)            # [128, 128]
    b1 = np.asarray(inputs["mlp_b1"], np.float32)            # [128]
    w2 = np.asarray(inputs["mlp_w2"], np.float32)            # [128, 1]
    b2 = np.asarray(inputs["mlp_b2"], np.float32)            # [1]

    # weight chunk tables: chunk 0 = rows 0:64 (x); chunk 1+s*3+k = rows
    # 64 + d*12 + s*3 + k (agg layout: ((d*4+s)*3+k)); prepacked host-side as
    # block-diagonal (both batches) bf16 lhsT tiles in one [128, 26*128] tensor
    Wc = np.zeros((NL, 13, D, D), np.float32)
    for li in range(NL):
        Wc[li, 0] = lw[li, :D]
        for s in range(4):
            for k in range(3):
                rows = 64 + np.arange(D) * 12 + s * 3 + k
                Wc[li, 1 + s * 3 + k] = lw[li, rows]
    wmm_host = np.zeros((128, NL * 13 * 128), np.float32)
    for li in range(NL):
        for c in range(13):
            o = (li * 13 + c) * 128
            wmm_host[:D, o:o + D] = Wc[li, c]
            wmm_host[D:, o + D:o + 128] = Wc[li, c]
    # readout MLP: hdn = relu(W1f^T . out_b + (q_b @ W1q + b1)); score = hdn @ w2
    # two stacked copies so each batch's rhs slice has a matching base partition
    w1f = np.zeros((128, 128), np.float32)
    w1f[:D, :] = w1[:D, :]
    w1f[D:, :] = w1[:D, :]

    relt = np.zeros((NL, NREL2, 128), np.float32)
    relt[:, :, :D] = rel
    relt[:, :, D:] = rel
    relt = relt.astype(nbf)

    # snap0-l0 shortcut constants: u_k = 1e-3 * sum_d W0[64 + d*12 + 9 + k]
    u3 = np.zeros((3, 128), np.float32)
    for k in range(3):
        rows = 64 + np.arange(D) * 12 + 9 + k
        u = 1e-3 * lw[0, rows].sum(axis=0)  # [D]
        u3[k, :D] = u
        u3[k, D:] = u
    u3[0, :D] += lb[0]
    u3[0, D:] += lb[0]
    u3l = u3.astype(nbf)  # [3, 128] lhsT
    # rhs scales [3, NPAD] in rank0 order (ones / scl / iscl)
    scl3 = np.stack([np.ones(NPAD), snaps[0]["v"][1].astype(np.float64),
                     snaps[0]["v"][2].astype(np.float64)]).astype(nbf)

    # per-core data
    per_core = []
    for c in range(NCORES):
        bsl = slice(c * BL, (c + 1) * BL)
        q = qw[r_i[bsl, 0]]                      # [BL, D] f32
        h_nodes = h_i[bsl, 0]
        # snap0 boundary (rank0-column-major): one-hot q columns
        init0 = np.zeros((128, NPAD), np.float32)
        for b in range(BL):
            r = int(snaps[0]["rank_of"][h_nodes[b]])
            init0[b * D:(b + 1) * D, r] += q[b]
        # snap0-l0 special output columns -> full-width mask + data tiles
        l0pos, l0vals = _l0_special_cols(
            snaps[0], h_nodes, q, rel[0], lw[0], lb[0],
            ei[0, 0], ei[0, 1], etp[0])
        l0mask = np.ones((128, NPAD), np.float32)  # complement mask
        l0data = np.zeros((128, NPAD), np.float32)
        for j, r in enumerate(l0pos):
            l0mask[:, r] = 0.0
            l0data[:, r] = l0vals[:, j]
        # snap1 indicator: scatter-add 0.5*q into tabB rows R0(h_b); dummy
        # row NPAD deduplicates when h_0 == h_1 (avoids RMW races)
        hidx = np.full(16, -1, np.int64)
        hadd = np.zeros((16, 128), np.float32)
        seen = {}
        for b in range(BL):
            hb = int(h_nodes[b])
            if hb in seen:
                j = seen[hb]
                hidx[b] = NPAD  # dummy scratch row, adds zeros
            else:
                seen[hb] = b
                j = b
                hidx[b] = int(r0w[hb])
            hadd[j, b * D:(b + 1) * D] += 0.5 * q[b]
        # per-batch MLP bias column: q_b @ W1[query rows] + b1
        mlpbias = (q @ w1[D:, :] + b1[None, :]).T.astype(np.float32)  # [128, BL]
        per_core.append(dict(
            init0=init0.astype(nbf),
            l0mask=l0mask.astype(nbf), l0data=l0data.astype(nbf),
            hidx_w=_wrap16(hidx), hadd=hadd.astype(nbf),
            mlpbias=mlpbias,
        ))

    return dict(
        snaps=snaps, Wc=Wc, relt=relt, lbias=lb, u3l=u3l, scl3=scl3,
        wmm_host=wmm_host.astype(nbf), w1f=w1f.astype(nbf),
        w2=w2.astype(nbf), b2=b2,
        per_core=per_core, h_i=h_i, t_i=t_i, r_i=r_i,
    )


def build(cfg, debug=()):
    """Build the Bass program. debug: iterable of dump names to add as outputs."""
    nc = bacc.Bacc("TRN2", target_bir_lowering=False, debug=False,
                   dynamic_dma_scratch_size=16384)
    snaps = cfg["snaps"]
    Lmax = max(snaps[0]["L"], snaps[1]["L"])

    # ---- DRAM tensors
    d_init0 = nc.dram_tensor("init0", [128, NPAD], BF16, kind="ExternalInput")
    d_l0mask = nc.dram_tensor("l0mask", [128, NPAD], BF16, kind="ExternalInput")
    d_l0data = nc.dram_tensor("l0data", [128, NPAD], BF16, kind="ExternalInput")
    d_hidx = nc.dram_tensor("hidx", [128, 1], I16, kind="ExternalInput")
    d_hadd = nc.dram_tensor("hadd", [16, 128], BF16, kind="ExternalInput")
    d_xidxB0 = nc.dram_tensor("xidxB0", list(snaps[0]["xidxB_w"].shape), I16,
                              kind="ExternalInput")
    d_xidxA1 = nc.dram_tensor("xidxA1", list(snaps[1]["xidxA_w"].shape), I16,
                              kind="ExternalInput")
    d_xidxB1 = nc.dram_tensor("xidxB1", list(snaps[1]["xidxB_w"].shape), I16,
                              kind="ExternalInput")
    d_etidx = [nc.dram_tensor(f"etidx{s}", list(snaps[s]["etidx_w"].shape), I16,
                              kind="ExternalInput") for s in range(HIST)]
    d_perm1 = nc.dram_tensor("permidx1", [128, NPAD // 16], I16, kind="ExternalInput")
    d_v = nc.dram_tensor("vvec", [HIST, 3, NPAD], BF16, kind="ExternalInput")
    d_u3 = nc.dram_tensor("u3", [3, 128], BF16, kind="ExternalInput")
    d_scl3 = nc.dram_tensor("scl3", [3, NPAD], BF16, kind="ExternalInput")
    d_relt = nc.dram_tensor("relt", [NL, NREL2, 128], BF16, kind="ExternalInput")
    d_wmm = nc.dram_tensor("wmm", [128, NL * 13 * 128], BF16, kind="ExternalInput")
    d_lb = nc.dram_tensor("lbias", [NL, D], F32, kind="ExternalInput")
    d_w1f = nc.dram_tensor("w1f", [128, 128], BF16, kind="ExternalInput")
    d_w2 = nc.dram_tensor("w2", [128, 1], BF16, kind="ExternalInput")
    d_mlpbias = nc.dram_tensor("mlpbias", [128, BL], F32, kind="ExternalInput")
    d_scores = nc.dram_tensor("scores", [BL, NPAD], BF16, kind="ExternalOutput")

    # work tables (node-major bf16, permuted row order): A = s0l0 out,
    # B = s0l1 out (pre-halved -> init1 after indicator scatter-add),
    # C = s1l0 out. tabB has 16 extra scratch rows (dedup dummy).
    d_tabs = {nm: nc.dram_tensor(nm, [NPAD + (16 if nm == "tabB" else 0), 128],
                                 BF16, kind="Internal")
              for nm in ("tabA", "tabB", "tabC")}

    dbg_tensors = {}

    def dbg(name, ap_like_shape, dtype):
        if name in debug:
            dbg_tensors[name] = nc.dram_tensor("dbg_" + name, ap_like_shape, dtype,
                                               kind="ExternalOutput")
            return dbg_tensors[name]
        return None

    def tab_write_ap(dram, h):
        # permuted-row transposed writeback: partition p owns rows
        # h*HNP + p*TPH + t (one contiguous TPH*256B run per partition)
        return bass.AP(tensor=dram, offset=h * HNP * 128,
                       ap=[[TPH * 128, 128], [128, TPH], [1, 128]])

    with tile.TileContext(nc) as tc, contextlib.ExitStack() as ctx:
        p_idx = ctx.enter_context(tc.tile_pool(name="idx", bufs=1))
        p_const = ctx.enter_context(tc.tile_pool(name="const", bufs=1))
        p_v = ctx.enter_context(tc.tile_pool(name="vrep", bufs=1))
        p_init = ctx.enter_context(tc.tile_pool(name="init", bufs=1))
        p_edge = ctx.enter_context(tc.tile_pool(name="edge", bufs=3))
        p_acc = ctx.enter_context(tc.tile_pool(name="acc", bufs=2))
        p_x = ctx.enter_context(tc.tile_pool(name="x", bufs=2))
        p_tr = ctx.enter_context(tc.tile_pool(name="tr", bufs=2))
        p_prod = ctx.enter_context(tc.tile_pool(name="prod", bufs=2))
        p_ps = ctx.enter_context(tc.tile_pool(name="ps", bufs=4, space="PSUM"))
        p_ps2 = ctx.enter_context(tc.tile_pool(name="ps2", bufs=2, space="PSUM"))
        p_misc = ctx.enter_context(tc.tile_pool(name="misc", bufs=1))

        nc.gpsimd.load_library(library_config.mlp)

        # ---- setup: layer-0 critical loads first
        u3sb = p_const.tile([3, 128], BF16, name="u3sb", tag="u3sb")
        nc.sync.dma_start(u3sb[:], d_u3.ap()[:])
        # scl3 borrows an xnext buffer (only rows 0:3 used; dead after layer 0)
        scl3full = p_x.tile([128, NPAD], BF16, name="scl3sb", tag="xnext")
        nc.sync.dma_start(scl3full[0:3, :], d_scl3.ap()[:])
        scl3sb = scl3full[0:3, :]
        # ---- setup: weights (prepacked block-diagonal lhsT tiles, one DMA)
        wmmall = p_const.tile([128, NL * 13 * 128], BF16, name="wmmall", tag="wmmall")
        nc.sync.dma_start(wmmall[:], d_wmm.ap()[:])
        wmm = {(li, c): wmmall[:, (li * 13 + c) * 128:(li * 13 + c + 1) * 128]
               for li in range(NL) for c in range(13)}
        w1fsb = p_const.tile([128, 128], BF16, name="w1fsb", tag="w1fsb")
        nc.sync.dma_start(w1fsb[:], d_w1f.ap()[:])
        w2sb = p_const.tile([128, 1], BF16, name="w2sb", tag="w2sb")
        nc.sync.dma_start(w2sb[:], d_w2.ap()[:])
        mlpb_sb = p_const.tile([128, BL], F32, name="mlpb", tag="mlpb")
        nc.sync.dma_start(mlpb_sb[:], d_mlpbias.ap()[:])
        # lbias columns: 0 = layer0, 1 = layer1, 2 = layer1 halved (tabB fold)
        lbsb = p_const.tile([128, NL + 1], F32, name="lbsb", tag="lbsb")
        for li in range(NL):
            nc.sync.dma_start(lbsb[:, li:li + 1], bass.AP(
                tensor=d_lb, offset=li * D, ap=[[0, 2], [1, D]]))
        nc.vector.tensor_single_scalar(out=lbsb[:, NL:NL + 1], in_=lbsb[:, 1:2],
                                       scalar=0.5, op=OP.mult)
        epssb = p_const.tile([128, 1], F32, name="epssb", tag="epssb")
        nc.vector.memset(epssb[:], EPS)
        hidx_sb = p_const.tile([128, 1], I16, name="hidx", tag="hidx")
        nc.sync.dma_start(hidx_sb[:], d_hidx.ap()[:])
        hadd_sb = p_const.tile([128, 1, 128], BF16, name="hadd", tag="hadd")
        nc.vector.memset(hadd_sb[:, 0, :], 0.0)
        nc.sync.dma_start(hadd_sb[0:16, 0, :], d_hadd.ap()[:])

        def generic_layer(s, li, xtab, xidx_sb, etidx_sb, initial,
                          vrep, x_in, outtab, evac_scale, evac_bias, mlp=False,
                          defer_cb=None, gather_dep=None):
            """One PNA layer; returns xnext SBUF tile."""
            sn = snaps[s]
            xnext = p_x.tile([128, NPAD], BF16, name="xnext", tag="xnext")
            for h in (0, 1):
                hsl = slice(h * HNP, (h + 1) * HNP)
                sb_scores = (p_misc.tile([1, BL * HNP], BF16, name="scores",
                                         tag="scores") if mlp else None)
                w0 = sn["w0h"][h]
                accs = {st: p_acc.tile([128, HNP], BF16, name=f"acc_{st}",
                                       tag=f"acc_{st}")
                        for st in ("sum", "max", "min", "sq")}
                # suffix init (indeg-0 ranks): acc = boundary
                if w0 < HNP:
                    for st in ("sum", "max", "min"):
                        nc.scalar.copy(accs[st][:, w0:],
                                       initial[:, h * HNP + w0:(h + 1) * HNP])
                    nc.scalar.activation(accs["sq"][:, w0:],
                                         initial[:, h * HNP + w0:(h + 1) * HNP],
                                         AF.Square)
                # ---- edge chunks (round-0 ops overwrite-copy so the chunk
                # pipeline does not wait for the boundary tile; the boundary
                # is merged just before stats-post)
                for ci, (c0, c1, ops) in enumerate(sn["chunks"][h]):
                    w = c1 - c0
                    relg = p_edge.tile([128, 1, CH], BF16, name="relg", tag="relg")
                    gi = nc.gpsimd.dma_gather(
                        out_ap=relg[:, :, :w],
                        in_ap=d_relt.ap()[li],
                        idxs_ap=etidx_sb[:, c0 // 16:c1 // 16],
                        num_idxs=w, num_idxs_reg=w, elem_size=128, transpose=True,
                        single_packet=False)
                    # keep prefetched rel gathers from cutting ahead of the
                    # previous layer's table writes on the serial DMA engines
                    _dep(gi, gather_dep, "relg after prev table write")
                    xg = p_edge.tile([128, 1, CH], BF16, name="xg", tag="xg")
                    xgi = nc.gpsimd.dma_gather(
                        out_ap=xg[:, :, :w],
                        in_ap=xtab.ap()[:],
                        idxs_ap=xidx_sb[:, c0 // 16:c1 // 16],
                        num_idxs=w, num_idxs_reg=w, elem_size=128, transpose=True,
                        single_packet=False)
                    if ci == 2 and h == 0 and defer_cb is not None:
                        defer_cb(xgi)
                        defer_cb = None
                    msg = xg[:, 0, :w]
                    nc.vector.tensor_tensor(out=msg, in0=msg, in1=relg[:, 0, :w],
                                            op=OP.mult)
                    for op_name, alu in (("sum", OP.add), ("max", OP.max),
                                         ("min", OP.min)):
                        acc = accs[op_name]
                        for (mo, ao, wd, r) in ops:
                            if r == 0:
                                nc.vector.tensor_copy(acc[:, ao:ao + wd],
                                                      msg[:, mo:mo + wd])
                            else:
                                nc.vector.tensor_tensor(
                                    out=acc[:, ao:ao + wd], in0=acc[:, ao:ao + wd],
                                    in1=msg[:, mo:mo + wd], op=alu)
                    nc.scalar.activation(msg, msg, AF.Square)
                    acc = accs["sq"]
                    for (mo, ao, wd, r) in ops:
                        if r == 0:
                            nc.vector.tensor_copy(acc[:, ao:ao + wd],
                                                  msg[:, mo:mo + wd])
                        else:
                            nc.vector.tensor_tensor(
                                out=acc[:, ao:ao + wd], in0=acc[:, ao:ao + wd],
                                in1=msg[:, mo:mo + wd], op=OP.add)
                if defer_cb is not None:
                    defer_cb(None)
                    defer_cb = None
                # ---- boundary (self-loop) merge
                if w0 > 0:
                    isl = slice(h * HNP, h * HNP + w0)
                    for st, alu in (("sum", OP.add), ("max", OP.max),
                                    ("min", OP.min)):
                        nc.vector.tensor_tensor(
                            out=accs[st][:, :w0], in0=accs[st][:, :w0],
                            in1=initial[:, isl], op=alu)
                    msq0 = p_init.tile([128, HNP], BF16, name="msq", tag="msq")
                    nc.scalar.activation(msq0[:, :w0], initial[:, isl], AF.Square)
                    nc.vector.tensor_tensor(
                        out=accs["sq"][:, :w0], in0=accs["sq"][:, :w0],
                        in1=msq0[:, :w0], op=OP.add)
                # ---- stats post: mean, std on [0, w0); suffix mean/max/min stay
                # as boundary (deg==1 -> invdeg==1), std suffix = sqrt(EPS)
                mean = accs["sum"]
                nc.vector.tensor_tensor(out=mean[:, :w0], in0=mean[:, :w0],
                                        in1=vrep[0][:, h * HNP:h * HNP + w0],
                                        op=OP.mult)
                sqm = accs["sq"]
                nc.vector.tensor_tensor(out=sqm[:, :w0], in0=sqm[:, :w0],
                                        in1=vrep[0][:, h * HNP:h * HNP + w0],
                                        op=OP.mult)
                msq = p_init.tile([128, HNP], BF16, name="msq", tag="msq")
                nc.scalar.activation(msq[:, :w0], mean[:, :w0], AF.Square)
                nc.vector.tensor_tensor(out=sqm[:, :w0], in0=sqm[:, :w0],
                                        in1=msq[:, :w0], op=OP.subtract)
                nc.scalar.activation(sqm[:, :w0], sqm[:, :w0], AF.Relu)
                nc.scalar.activation(sqm[:, :w0], sqm[:, :w0], AF.Sqrt,
                                     bias=epssb[:, 0:1])
                if w0 < HNP:
                    nc.vector.memset(sqm[:, w0:], 1e-3)
                stat_t = {"mean": mean, "max": accs["max"], "min": accs["min"],
                          "std": sqm}
                # ---- matmul: tile-major over node tiles
                tout = p_tr.tile([128, HNP // 128, 128], BF16, name="tout",
                                 tag="tout")
                for t in range(HNP // NTILE):
                    tsl = slice(h * HNP + t * NTILE, h * HNP + (t + 1) * NTILE)
                    asl = slice(t * NTILE, (t + 1) * NTILE)
                    prods = {}
                    for si, stn in enumerate(STATS):
                        for k in (1, 2):
                            pr = p_prod.tile([128, NTILE], BF16,
                                             name=f"pr_{si}_{k}", tag=f"pr_{si}_{k}")
                            nc.vector.tensor_tensor(
                                out=pr[:], in0=stat_t[stn][:, asl],
                                in1=vrep[k][:, tsl], op=OP.mult)
                            prods[(si, k)] = pr
                    pst = p_ps.tile([128, NTILE], F32, name="pst", tag="pst")
                    for c in range(13):
                        if c == 0:
                            rhs = x_in[:, tsl]
                        else:
                            si, k = (c - 1) // 3, (c - 1) % 3
                            if k == 0:
                                rhs = stat_t[STATS[si]][:, asl]
                            else:
                                rhs = prods[(si, k)][:]
                        nc.tensor.matmul(out=pst[:], lhsT=wmm[(li, c)][:],
                                         rhs=rhs, start=(c == 0), stop=(c == 12))
                    nc.scalar.activation(xnext[:, tsl], pst[:], AF.Relu,
                                         scale=evac_scale, bias=evac_bias)
                    tpt = NTILE // 128
                    if outtab is not None:
                        # per-tile transpose: shortens the write tail
                        nc.sync.dma_start_transpose(
                            tout[:, t * tpt:(t + 1) * tpt, :], xnext[:, tsl])
                    if mlp:
                        # readout: hdn_b = relu(W1f^T out_b + bias_b),
                        # score_b = hdn_b @ w2 -> all-node scores
                        for b in range(BL):
                            psh = p_ps2.tile([128, NTILE], F32, name="mlp1",
                                             tag="mlp1")
                            nc.tensor.matmul(out=psh[:],
                                             lhsT=w1fsb[b * D:(b + 1) * D, :],
                                             rhs=xnext[b * D:(b + 1) * D, tsl],
                                             start=True, stop=True)
                            hd = p_misc.tile([128, NTILE], BF16, name="hd",
                                             tag="hd", bufs=2)
                            nc.scalar.activation(hd[:], psh[:], AF.Relu,
                                                 bias=mlpb_sb[:, b:b + 1])
                            pss = p_ps2.tile([1, NTILE], F32, name="mlp2",
                                             tag="mlp2")
                            nc.tensor.matmul(out=pss[:], lhsT=w2sb[:], rhs=hd[:],
                                             start=True, stop=True)
                            nc.scalar.activation(
                                sb_scores[0:1, b * HNP + t * NTILE:
                                          b * HNP + (t + 1) * NTILE], pss[:],
                                AF.Copy)
                # per-half transposed writeback (permuted row order)
                if outtab is not None:
                    wr_inst = nc.sync.dma_start(tab_write_ap(outtab, h), tout[:])
                if mlp:
                    for b in range(BL):
                        nc.sync.dma_start(
                            d_scores.ap()[b:b + 1, hsl],
                            sb_scores[0:1, b * HNP:(b + 1) * HNP])
            if (t_ := dbg(f"x_s{s}_l{li}", [128, NPAD], BF16)) is not None:
                nc.sync.dma_start(t_.ap()[:], xnext[:])
            return xnext, (wr_inst if outtab is not None else None)

        # ================= snap 0 =================
        sn0 = snaps[0]
        xidxB0_sb = p_idx.tile([128, Lmax // 16], I16, name="xidxB0", tag="xidxB")
        nc.sync.dma_start(xidxB0_sb[:, :sn0["L"] // 16], d_xidxB0.ap()[:])
        etidx0_sb = p_idx.tile([128, sn0["L"] // 16], I16, name="etidx0", tag="etidx0")
        nc.sync.dma_start(etidx0_sb[:], d_etidx[0].ap()[:])
        # ---- snap0 layer0: xnext = relu(u0 + scl*u1 + iscl*u2), special cols fixed
        xnext0 = p_x.tile([128, NPAD], BF16, name="xnext", tag="xnext")
        for h in (0, 1):
            hsl = slice(h * HNP, (h + 1) * HNP)
            l0mask_sb = p_acc.tile([128, HNP], BF16, name="l0mask", tag="acc_sum")
            nc.sync.dma_start(l0mask_sb[:], d_l0mask.ap()[:, hsl])
            l0data_sb = p_acc.tile([128, HNP], BF16, name="l0data", tag="acc_max")
            nc.sync.dma_start(l0data_sb[:], d_l0data.ap()[:, hsl])
            for t in range(HNP // NTILE):
                tsl = slice(h * HNP + t * NTILE, h * HNP + (t + 1) * NTILE)
                pst = p_ps.tile([128, NTILE], F32, name="pst", tag="pst")
                nc.tensor.matmul(out=pst[:], lhsT=u3sb[:], rhs=scl3sb[:, tsl],
                                 start=True, stop=True)
                nc.scalar.activation(xnext0[:, tsl], pst[:], AF.Relu)
            # xnext = xnext * (1 - mask) + data (mask holds the complement)
            nc.vector.tensor_tensor(out=xnext0[:, hsl], in0=xnext0[:, hsl],
                                    in1=l0mask_sb[:], op=OP.mult)
            nc.vector.tensor_tensor(out=xnext0[:, hsl], in0=xnext0[:, hsl],
                                    in1=l0data_sb[:], op=OP.add)
            tout = p_tr.tile([128, HNP // 128, 128], BF16, name="tout", tag="tout")
            nc.sync.dma_start_transpose(tout[:], xnext0[:, hsl])
            wrA = nc.sync.dma_start(tab_write_ap(d_tabs["tabA"], h), tout[:])
        if (t_ := dbg("x_s0_l0", [128, NPAD], BF16)) is not None:
            nc.sync.dma_start(t_.ap()[:], xnext0[:])
        x_prev = xnext0

        vrep0 = [p_v.tile([128, NPAD], BF16, name=f"v0{j}", tag=f"v{j}")
                 for j in range(3)]
        initial0_t = p_init.tile([128, 1, NPAD], BF16, name="initial0", tag="initial")
        initial0 = initial0_t[:, 0, :]

        sn1 = snaps[1]
        xidxA1_sb = p_idx.tile([128, sn1["L"] // 16], I16, name="xidxA1", tag="xidxA1")
        xidxB1_sb = p_idx.tile([128, Lmax // 16], I16, name="xidxB1", tag="xidxB")
        etidx1_sb = p_idx.tile([128, sn1["L"] // 16], I16, name="etidx1", tag="etidx1")
        perm1_sb = p_idx.tile([128, NPAD // 16], I16, name="perm1", tag="perm1")

        def load_s0_deferred(dep):
            # boundary + v vectors load behind the first chunk gathers so the
            # tabA writes and gathers lead on the serial DMA engines
            _dep(nc.sync.dma_start(initial0_t[:, 0, :], d_init0.ap()[:]),
                 dep, "init0 load after early gathers")
            for j, t in enumerate(vrep0):
                _dep(nc.sync.dma_start(t[:], bass.AP(
                    tensor=d_v, offset=(0 * 3 + j) * NPAD, ap=[[0, 128], [1, NPAD]])),
                     dep, "vrep0 load after early gathers")
            _dep(nc.sync.dma_start(xidxA1_sb[:], d_xidxA1.ap()[:]),
                 dep, "s1 idx load after early gathers")
            _dep(nc.sync.dma_start(xidxB1_sb[:, :sn1["L"] // 16], d_xidxB1.ap()[:]),
                 dep, "s1 idx load after early gathers")
            _dep(nc.sync.dma_start(etidx1_sb[:], d_etidx[1].ap()[:]),
                 dep, "s1 idx load after early gathers")
            _dep(nc.sync.dma_start(perm1_sb[:], d_perm1.ap()[:]),
                 dep, "s1 idx load after early gathers")
            if (t_ := dbg("initial0", [128, NPAD], BF16)) is not None:
                nc.sync.dma_start(t_.ap()[:], initial0)


        # ---- snap0 layer1 (evac pre-halved into tabB)
        x_prev, wrB = generic_layer(0, 1, d_tabs["tabA"], xidxB0_sb, etidx0_sb,
                                    initial0, vrep0, x_prev, d_tabs["tabB"],
                                    0.5, lbsb[:, NL:NL + 1],
                                    defer_cb=load_s0_deferred)

        # ---- indicator scatter-add: tabB += 0.5*q at rows R0(h_b) -> init1
        scat = nc.gpsimd.dma_scatter_add(
            out_ap=d_tabs["tabB"].ap()[:], in_ap=hadd_sb[:],
            idxs_ap=hidx_sb[:], num_idxs=16, num_idxs_reg=BL, elem_size=128,
            single_packet=False)

        # ================= snap 1 =================
        vrep1 = [p_v.tile([128, NPAD], BF16, name=f"v1{j}", tag=f"v{j}")
                 for j in range(3)]
        # initial1 = perm-gather(tabB) (tabB == init1 after scatter-add);
        # issued after the first edge-chunk gathers so they lead on DMA
        initial1_t = p_init.tile([128, 1, NPAD], BF16, name="initial1",
                                 tag="initial")
        initial1 = initial1_t[:, 0, :]

        def fill_initial1(dep):
            nc.gpsimd.dma_gather(
                out_ap=initial1_t[:], in_ap=d_tabs["tabB"].ap()[:],
                idxs_ap=perm1_sb[:], num_idxs=NPAD, num_idxs_reg=NPAD,
                elem_size=128, transpose=True, single_packet=False)
            for j, t in enumerate(vrep1):
                _dep(nc.sync.dma_start(t[:], bass.AP(
                    tensor=d_v, offset=(1 * 3 + j) * NPAD, ap=[[0, 128], [1, NPAD]])),
                     dep, "vrep1 load after early gathers")
            if (t_ := dbg("initial1", [128, NPAD], BF16)) is not None:
                nc.sync.dma_start(t_.ap()[:], initial1)

        x_prev, wrC = generic_layer(1, 0, d_tabs["tabB"], xidxA1_sb, etidx1_sb,
                                    initial1, vrep1, initial1, d_tabs["tabC"],
                                    1.0, lbsb[:, 0:1], defer_cb=fill_initial1)
        x_prev, _ = generic_layer(1, 1, d_tabs["tabC"], xidxB1_sb, etidx1_sb,
                                  initial1, vrep1, x_prev, None,
                                  1.0, lbsb[:, 1:2], mlp=True)

    nc.compile()
    return nc, dbg_tensors


def make_in_maps(cfg):
    shared = {}
    shared["xidxB0"] = cfg["snaps"][0]["xidxB_w"]
    shared["xidxA1"] = cfg["snaps"][1]["xidxA_w"]
    shared["xidxB1"] = cfg["snaps"][1]["xidxB_w"]
    for s in range(HIST):
        shared[f"etidx{s}"] = cfg["snaps"][s]["etidx_w"]
    shared["permidx1"] = cfg["snaps"][1]["perm_w"]
    shared["vvec"] = np.stack([cfg["snaps"][s]["v"] for s in range(HIST)])
    shared["u3"] = cfg["u3l"]
    shared["scl3"] = cfg["scl3"]
    shared["relt"] = cfg["relt"]
    shared["wmm"] = cfg["wmm_host"]
    shared["lbias"] = cfg["lbias"]
    shared["w1f"] = cfg["w1f"]
    shared["w2"] = cfg["w2"]
    in_maps = []
    for c in range(NCORES):
        pc = cfg["per_core"][c]
        in_maps.append(dict(
            shared,
            init0=pc["init0"], l0mask=pc["l0mask"], l0data=pc["l0data"],
            hidx=pc["hidx_w"], hadd=pc["hadd"], mlpbias=pc["mlpbias"],
        ))
    return in_maps


_CACHE = {}


def _get_program(cfg, key):
    if key not in _CACHE:
        _CACHE[key] = build(cfg)
    return _CACHE[key]


def kernel(**inputs):
    cfg = preprocess(inputs)
    key = (np.asarray(inputs["edge_index"]).tobytes(),
           np.asarray(inputs["edge_type"]).tobytes(),
           np.asarray(inputs["query_triple"]).tobytes())
    nc, _ = _get_program(cfg, key)
    in_maps = make_in_maps(cfg)
    res = bass_utils.run_bass_kernel_spmd(nc, in_maps, core_ids=list(range(NCORES)))
    full = np.concatenate([res.results[c]["scores"] for c in range(NCORES)],
                          axis=0)  # [B, NPAD] per-node scores, rank1 columns
    t_cols = cfg["snaps"][1]["rank_of"][cfg["t_i"]]  # [B, NEG]
    b2 = float(np.asarray(inputs["mlp_b2"]).reshape(-1)[0])
    scores = full[np.arange(B)[:, None], t_cols] + b2
    return scores.astype(np.float32)
